# revision 1
# baseline (speedup 1.0000x reference)
"""Causal attention (QKV proj + softmax(QK^T/sqrt(d))V) on 8 TRN2 NeuronCores.

Sharding: data-parallel over batch (B=8, one batch element per core).
Per-core kernel, all matmuls in float32r (fast fp32 mode):
  phase 0: x [T,D] -> x^T stored t-block-major [P, tb, dc, 128] via PE
           transposes (SBUF resident)
  phase 1: Q^T -> DRAM scratch (reloaded per supertile); K^T -> SBUF
           resident; then V = x @ Wv evicted IN PLACE over the x^T slab
           of the same t-block (x^T morphs into resident V, no roundtrip)
  phase 2: per 512-wide query supertile: S^T = K Q^T (so softmax probs are
           produced directly in the lhsT layout needed by P@V), exp on ACT
           with fused 1/sqrt(D) scale, causal mask on the diagonal band,
           P@V with interleaved ones-matmul row sums on PE, reciprocal
           normalize, store.

DMA rings: sync = streaming loads; scalar = stores; gpsimd (SWDGE) also
carries odd x tiles and Q-tile prefetches so they issue at readiness.
"""

import numpy as np

T = 2048
D = 1024
E = 1024
N_CORES = 8
P = 128
TS = 512  # t-slice / supertile width
SCALE = 1.0 / 32.0  # 1/sqrt(D)

DC = D // P  # 8 d-chunks
EC = E // P  # 8 e-chunks
TB = T // P  # 16 t-blocks of 128
NTS = T // TS  # 4 t-slices of 512
JB = TS // P  # 4 q-blocks per supertile
HB = TB // 2  # pT half size in k-blocks
QB = TB // 4  # pT quarter size in k-blocks


def _attention_kernel(ctx, tc, out, x, wq, wk, wv):
    import concourse.bass as bass
    from concourse import mybir
    from concourse.bass import ts
    from concourse.masks import make_identity

    nc = tc.nc
    f32 = mybir.dt.float32
    f32r = mybir.dt.float32r
    AF = mybir.ActivationFunctionType

    # ---- DRAM scratch ----
    dram = ctx.enter_context(tc.tile_pool(name="dram", bufs=1, space="DRAM"))
    qdram = dram.tile([EC, P, T], f32r)  # Q^T[e,t], e = ec*128 + ep

    # ---- left-side SBUF pools ----
    const = ctx.enter_context(tc.tile_pool(name="const", bufs=1))
    ones_f32 = const.tile([P, 2], f32)
    nc.vector.memset(ones_f32[:], 1.0)
    ones_col = const.tile([P, 2], f32r)
    nc.vector.tensor_copy(ones_col[:], ones_f32[:])
    # warm the ACT exp table set at program start (off the critical path)
    exp_warm = const.tile([P, 2], f32)
    nc.scalar.activation(exp_warm[:], ones_f32[:], AF.Exp)
    identity_f32 = const.tile([P, P], f32)
    make_identity(nc, identity_f32[:])
    identity = const.tile([P, P], f32r)
    nc.vector.tensor_copy(identity[:], identity_f32[:])

    kt_pool = ctx.enter_context(tc.tile_pool(name="ktres", bufs=1))
    KT = kt_pool.tile([P, EC, T], f32r)  # K^T[e, t], e = ec*128 + ep

    # One wide causal mask; mask_j = wide[:, 384-128j : 384-128j+512] keeps
    # entries where f - p - 128*j >= 0 in S^T coords (p=key, f=query).
    mask_pool = ctx.enter_context(tc.tile_pool(name="maskp", bufs=1))
    WIDE = TS + (JB - 1) * P
    wide_f32 = mask_pool.tile([P, WIDE], f32)
    nc.gpsimd.memset(wide_f32[:], 1.0)
    nc.gpsimd.affine_select(
        out=wide_f32[:],
        in_=wide_f32[:],
        compare_op=mybir.AluOpType.is_ge,
        fill=0.0,
        base=-(JB - 1) * P,
        pattern=[[1, WIDE]],
        channel_multiplier=-1,
    )
    wide = mask_pool.tile([P, WIDE], f32r)
    nc.vector.tensor_copy(wide[:], wide_f32[:])
    masks = [
        wide[:, (JB - 1) * P - P * j : (JB - 1) * P - P * j + TS]
        for j in range(JB)
    ]

    # supertile-0 Q tiles prefetch here (left side, not gated by reuse)
    qt0_pool = ctx.enter_context(tc.tile_pool(name="qt0", bufs=10))

    # ---- right-side work pools ----
    tc.swap_default_side()
    xv_pool = ctx.enter_context(tc.tile_pool(name="xv", bufs=1))
    # x^T t-block-major; after phase 1 each slab is overwritten in place
    # with V[tb] so this same tile is the resident V in phase 2.
    xv = xv_pool.tile([P, TB, DC, P], f32r)  # [dp, tb, dc, tl]
    Vres = xv[:].rearrange("p tb dc e -> p tb (dc e)")  # V[t, e] view
    qstg = tc.alloc_tile_pool(name="qstg", bufs=3)
    wqk_pool = tc.alloc_tile_pool(name="wqk", bufs=3)
    wvh_pool = tc.alloc_tile_pool(name="wvh", bufs=2)
    tc.swap_default_side()

    # ---- PSUM pools for phases 0/1 ----
    ps_tp = tc.alloc_tile_pool(name="ps_tp", bufs=2, space="PSUM")
    ps_proj = tc.alloc_tile_pool(name="ps_proj", bufs=6, space="PSUM")

    wq_view0 = wq.bitcast(f32r).rearrange("(dc dp) e -> dp dc e", dp=P)
    wr_q01 = []
    for eb in range(2):
        wr = wqk_pool.tile([P, DC, P], f32r, tag="wqk", name=f"wrq_{eb}")
        eng = nc.gpsimd if eb == 0 else nc.scalar
        eng.dma_start(wr[:], wq_view0[:, :, ts(eb, P)])
        wr_q01.append(wr)

    # ===== phase 0: x -> x^T via PE transposes (f32r: 1.5 cyc/row) =====
    for tb in range(TB):
        xa = wvh_pool.tile([P, D], f32r, tag="wvh", name=f"xa_{tb}")
        eng = (nc.sync, nc.gpsimd, nc.scalar)[tb % 3]
        if tb < 2:
            # per-dc loads so the first transposes start ~4x earlier
            for dc in range(DC):
                eng.dma_start(
                    xa[:, ts(dc, P)], x[ts(tb, P), ts(dc, P)].bitcast(f32r)
                )
        else:
            eng.dma_start(
                xa[:, 0 : D // 2], x[ts(tb, P), 0 : D // 2].bitcast(f32r)
            )
            eng.dma_start(
                xa[:, D // 2 : D], x[ts(tb, P), D // 2 : D].bitcast(f32r)
            )
        for dc in range(DC):
            pt = ps_tp.tile([P, P], f32r)
            nc.tensor.transpose(pt[:], xa[:, ts(dc, P)], identity[:])
            # evict + round to f32r; alternate engines
            if dc % 2 == 0:
                nc.vector.tensor_copy(xv[:, tb, dc, :], pt[:])
            else:
                nc.scalar.copy(xv[:, tb, dc, :], pt[:])

    # ======== phase 1a: Q^T (to DRAM scratch), K^T (resident) ========
    # Supertile 0's Q tiles (tsl=0) are evicted straight into SBUF,
    # skipping the DRAM roundtrip entirely.
    qts0 = [
        qt0_pool.tile([P, TS], f32r, tag="qt0", name=f"qts0_{ec}")
        for ec in range(EC)
    ]
    for w_ap, is_q in ((wq, True), (wk, False)):
        w_view = w_ap.bitcast(f32r).rearrange("(dc dp) e -> dp dc e", dp=P)
        for eb in range(EC):
            if is_q and eb < 2:
                wr = wr_q01[eb]
            else:
                wr = wqk_pool.tile([P, DC, P], f32r, tag="wqk")
                nc.sync.dma_start(wr[:], w_view[:, :, ts(eb, P)])
            for tsl in range(NTS):
                pp = ps_proj.tile([P, TS], f32)
                for dc in range(DC):
                    nc.tensor.matmul(
                        pp[:],
                        wr[:, dc, :],
                        xv[:, 4 * tsl : 4 * tsl + 4, dc, :],
                        start=(dc == 0),
                        stop=(dc == DC - 1),
                    )
                if is_q and tsl == 0:
                    nc.vector.tensor_copy(qts0[eb][:], pp[:])
                elif is_q:
                    qst = qstg.tile([P, TS], f32r, tag="qstage")
                    nc.vector.tensor_copy(qst[:], pp[:])
                    nc.scalar.dma_start(qdram[eb, :, ts(tsl, TS)], qst[:])
                elif tsl % 2 == 0:
                    nc.vector.tensor_copy(KT[:, eb, ts(tsl, TS)], pp[:])
                else:
                    nc.scalar.copy(KT[:, eb, ts(tsl, TS)], pp[:])

    # ========== phase 1b: V = x @ Wv, evicted in place over x^T ==========
    # tb-outer with both Wv halves resident: both psums must be computed
    # before the in-place evicts may overwrite this t-block's x^T slab.
    wv_view = wv.bitcast(f32r).rearrange("(dc dp) e -> dp dc e", dp=P)
    wvhs = []
    for eh in range(E // TS):
        wvh = wvh_pool.tile([P, DC, TS], f32r, tag="wvh", name=f"wvh_{eh}")
        nc.sync.dma_start(wvh[:], wv_view[:, :, ts(eh, TS)])
        wvhs.append(wvh)
    for tb in range(TB):
        pps = []
        for eh in range(E // TS):
            pp = ps_proj.tile([P, TS], f32)
            for dc in range(DC):
                nc.tensor.matmul(
                    pp[:],
                    xv[:, tb, dc, :],
                    wvhs[eh][:, dc, :],
                    start=(dc == 0),
                    stop=(dc == DC - 1),
                )
            pps.append(pp)
        # in-place evicts over the x^T slab of this t-block (WAR: both
        # psum groups above have read the slab before these run)
        nc.scalar.copy(Vres[:, tb, ts(0, TS)], pps[0][:])
        nc.vector.tensor_copy(Vres[:, tb, ts(1, TS)], pps[1][:])

    wvh_pool.release()
    wqk_pool.release()
    qstg.release()
    ps_proj.release()
    ps_tp.release()

    # ================= phase 2: attention =================
    ps_s = tc.alloc_tile_pool(name="ps_s", bufs=4, space="PSUM")
    ps_o = tc.alloc_tile_pool(name="ps_o", bufs=2, space="PSUM")
    ps_sum = tc.alloc_tile_pool(name="ps_sum", bufs=2, space="PSUM")

    tc.swap_default_side()
    pt_pool = ctx.enter_context(tc.tile_pool(name="pt", bufs=5))
    rs_pool = ctx.enter_context(tc.tile_pool(name="rs", bufs=8))
    ostg = ctx.enter_context(tc.tile_pool(name="ostg", bufs=3))
    tc.swap_default_side()

    for sup in range(NTS):
        nkb = JB * sup + JB  # key blocks 0..nkb-1
        if sup == 0:
            qts = qts0
        else:
            qts = []
            for ec in range(EC):
                q1 = qt0_pool.tile([P, TS], f32r, tag="qt0", name=f"qt_{sup}_{ec}")
                eng = nc.sync if ec % 2 == 0 else nc.gpsimd
                eng.dma_start(q1[:], qdram[ec, :, ts(sup, TS)])
                qts.append(q1)
        pt_parts = [
            pt_pool.tile([P, QB, TS], f32r, tag="pt", name=f"ptp_{sup}_0")
        ]

        # --- S^T blocks + exp + causal mask ---
        for k in range(nkb):
            ssp = ps_s.tile([P, TS], f32)
            for ec in range(EC):
                nc.tensor.matmul(
                    ssp[:],
                    KT[:, ec, ts(k, P)],
                    qts[ec][:],
                    start=(ec == 0),
                    stop=(ec == EC - 1),
                )
            if k // QB >= len(pt_parts):
                pt_parts.append(
                    pt_pool.tile(
                        [P, QB, TS], f32r, tag="pt",
                        name=f"ptp_{sup}_{k // QB}",
                    )
                )
            pk = pt_parts[k // QB][:, k % QB, :]
            nc.scalar.activation(pk[:], ssp[:], AF.Exp, scale=SCALE)
            j = k - JB * sup
            if j >= 0:
                nc.vector.tensor_mul(pk[:], pk[:], masks[j])

        # --- P @ V (+ row sums interleaved in eh=0), normalize, store ---
        rss = {}
        for eh in range(E // TS):
            for jq in range(JB):
                qb = JB * sup + jq
                nk = qb + 1
                po = ps_o.tile([P, TS], f32)
                if eh == 0:
                    pos = ps_sum.tile([P, 2], f32)
                for k in range(nk):
                    lhsT = pt_parts[k // QB][:, k % QB, ts(jq, P)]
                    nc.tensor.matmul(
                        po[:],
                        lhsT,
                        Vres[:, k, ts(eh, TS)],
                        start=(k == 0),
                        stop=(k == nk - 1),
                    )
                    if eh == 0:
                        nc.tensor.matmul(
                            pos[:],
                            lhsT,
                            ones_col[:],
                            start=(k == 0),
                            stop=(k == nk - 1),
                        )
                if eh == 0:
                    rs = rs_pool.tile(
                        [P, 1], f32, tag="rs", name=f"rs_{sup}_{jq}"
                    )
                    nc.vector.reciprocal(rs[:], pos[:, 0:1])
                    rss[jq] = rs
                ost = ostg.tile([P, TS], f32, tag="ostage")
                nc.scalar.activation(
                    ost[:], po[:], AF.Copy, scale=rss[jq][:]
                )
                nc.scalar.dma_start(out[ts(qb, P), ts(eh, TS)], ost[:])

    ps_sum.release()
    ps_o.release()
    ps_s.release()


def build_program():
    from contextlib import ExitStack

    import concourse.bacc as bacc
    import concourse.tile as tile
    from concourse import mybir

    nc = bacc.Bacc("TRN2", target_bir_lowering=False, debug=False)
    f32 = mybir.dt.float32
    x = nc.dram_tensor("x", [T, D], f32, kind="ExternalInput").ap()
    wq = nc.dram_tensor("Wq", [D, E], f32, kind="ExternalInput").ap()
    wk = nc.dram_tensor("Wk", [D, E], f32, kind="ExternalInput").ap()
    wv = nc.dram_tensor("Wv", [D, E], f32, kind="ExternalInput").ap()
    out = nc.dram_tensor("out", [T, E], f32, kind="ExternalOutput").ap()

    with tile.TileContext(nc) as tc:
        with ExitStack() as ctx:
            _attention_kernel(ctx, tc, out, x, wq, wk, wv)
    nc.compile()
    return nc


def kernel(x, Wq, Wk, Wv, _trace=False):
    from concourse.bass_utils import run_bass_kernel_spmd

    x = np.ascontiguousarray(np.asarray(x), dtype=np.float32)
    Wq = np.ascontiguousarray(np.asarray(Wq), dtype=np.float32)
    Wk = np.ascontiguousarray(np.asarray(Wk), dtype=np.float32)
    Wv = np.ascontiguousarray(np.asarray(Wv), dtype=np.float32)
    assert x.shape == (N_CORES, T, D), x.shape

    nc = build_program()
    in_maps = [
        {"x": np.ascontiguousarray(x[b]), "Wq": Wq, "Wk": Wk, "Wv": Wv}
        for b in range(N_CORES)
    ]
    last_err = None
    for attempt in range(3):
        try:
            res = run_bass_kernel_spmd(
                nc, in_maps, core_ids=list(range(N_CORES)), trace=_trace
            )
            break
        except Exception as e:  # transient device wedge: retry
            last_err = e
            import time

            time.sleep(5.0 * (attempt + 1))
    else:
        raise last_err
    out = np.stack([res.results[b]["out"] for b in range(N_CORES)], axis=0)
    if _trace:
        kernel.last_results = res
    return out


kernel.last_results = None

